# revision 33
# baseline (speedup 1.0000x reference)
"""Trainium2 Bass kernel for the Noisy-Weights BNN MLP.

Computation (full problem):
  noise1[0] = 0;  W1n = W1[None] + noise1            # [16, 512, 512]
  X = sigmoid(A @ W0)        A = batch.reshape(2048, 784)
  Y_s = sigmoid(X @ W1n[s])
  Z_s = sigmoid(Y_s @ W2)    -> out [16, 32, 64, 10]

Sharding over 8 NeuronCores: 2 replica-groups (8 replicas each) x
4 token-groups (512 tokens each).  Each core redundantly computes the
shared layer 0 for its 512 tokens, then its 8 replicas of layers 1+2.

On-device layout: every matmul is a native out = lhsT.T @ rhs with the
contraction dim on SBUF partitions:
  layer0: lhsT = W0 tile, rhs = A^T tile -> psum X^T, sigmoid -> fp8
  layer1: lhsT = W1n tile, rhs = X^T     -> psum Y^T, sigmoid -> fp8
  layer2: lhsT = W2 tile [128k, 16pad], rhs = Y^T, two DoubleRow
          k-pair matmuls -> psum Z^T logits, one DVE copy to SBUF
Host applies the final sigmoid + transpose (tiny: 1.3 MB total).

Precision: all three layers run fp8e4m3 with DoubleRow perf mode
(2 k-tiles per matmul, 2x PE throughput when warm, half the DMA
bytes).  numpy-model rel-L2 vs the fp32 reference: 1.0e-2 (gate is
2e-2).  fp32 PSUM accumulation throughout.

Schedule notes: a few N=512 dummy matmuls warm the PE clock (HAM)
while the first DMA chunk lands; layer-0 A^T/W0 are packed in k-tile
pair blocks and DMA'd in chunks so compute starts after the first
256 KB; each replica's layer-2 k-pairs are issued one sigmoid behind
(pair mp in slot mp of the NEXT replica) so the PE FIFO never stalls
behind a not-yet-ready activation.  Steady state runs both TensorE
and ScalarE at ~100% occupancy (~2.0 us per replica).
"""

import os
import sys

import numpy as np
import ml_dtypes

if "/opt/trn_rl_repo" not in sys.path:
    sys.path.insert(0, "/opt/trn_rl_repo")

import concourse.bass as bass  # noqa: E402
import concourse.tile as tile  # noqa: E402
from concourse import bacc, mybir  # noqa: E402
from concourse.bass_utils import run_bass_kernel_spmd  # noqa: E402

# ---- problem constants (hardcoded; kernel.py must be self-contained) ----
S = 16           # noisy-weight replicas
BT = 2048        # batch tokens = 32 * 64
D_IN = 784
D_H = 512
D_OUT = 10
KA = 896         # 784 zero-padded to 7 * 128
N_CORES = 8
SG = 2           # replica groups
TG = 4           # token groups
R_LOC = S // SG          # replicas per core = 8
NT = BT // TG            # tokens per core = 512
KA_T = KA // 128         # 7 k-tiles for layer 0
KH_T = D_H // 128        # 4 k-tiles / m-tiles for hidden dims
AW_K = NT + D_H          # interleaved A^T|W0 stride per k-tile = 1024
W2C = 16                 # W2 k-tile columns: 10 outputs padded to 16 so the
                         # DoubleRow weight AP stride is 16 B-aligned

BF16 = mybir.dt.bfloat16
FP8 = mybir.dt.float8e4
F32 = mybir.dt.float32
DR = mybir.MatmulPerfMode.DoubleRow

# Dummy matmuls covering the first input-DMA wait (~7.5us -> ~10.4us: DMA
# can't start before the NEFF preamble ends and its completion semaphore
# takes ~0.75us after the data lands).  Keeping the PE busy the whole time
# both avoids the idle gap and lets the HAM clock gate reach 2.4 GHz
# before the first real matmul — cold DR matmuls are 2x slower.  N=512
# streaming matmuls (~85% duty) are needed to register as "busy" with
# the HAM activity window; short N=128 ones (~50% duty) leave it cold.
N_WARM = 8

_CACHE = {}

last_results = None  # BassKernelResults of the most recent run (for test.py)


def _build_program():
    """One SPMD Bass program; per-core differences live entirely in data."""
    nc = bacc.Bacc(None, target_bir_lowering=False, debug=False,
                   enable_partition_id=False)

    # layer-0 inputs interleaved per k-tile: aw[:, k*1024+0:512] = A^T k-tile,
    # aw[:, k*1024+512:1024] = W0 k-tile
    aw_d = nc.dram_tensor("aw_pack", [128, KA_T * AW_K], FP8,
                          kind="ExternalInput")
    w1_d = nc.dram_tensor("w1_pack", [128, R_LOC * KH_T * D_H], FP8,
                          kind="ExternalInput")
    w2_d = nc.dram_tensor("w2_pack", [128, KH_T * W2C], FP8,
                          kind="ExternalInput")
    zt_d = nc.dram_tensor("zt", [D_OUT, R_LOC * NT], F32, kind="ExternalOutput")

    SIG = mybir.ActivationFunctionType.Sigmoid
    AW_CHUNKS = [(0, 2), (2, 4), (4, 6)]   # full k-tile ranges per chunk
    K6 = (KA_T - 1) * AW_K                 # col offset of the 16-row k-tile 6

    with tile.TileContext(nc) as tc:
        with (
            tc.tile_pool(name="consts", bufs=1) as consts,
            tc.tile_pool(name="w1p", bufs=1) as w1p,
            tc.tile_pool(name="yp", bufs=3) as yp,
            tc.tile_pool(name="px", bufs=3, space="PSUM") as px,
            tc.tile_pool(name="pz", bufs=2, space="PSUM") as pz,
        ):
            warm_sb = consts.tile([128, 512], BF16)
            aw_sb = consts.tile([128, KA_T * AW_K], FP8)
            w2_sb = consts.tile([128, KH_T * W2C], FP8)
            x_sb = consts.tile([128, KH_T * NT], FP8)
            z_sb = consts.tile([D_OUT, R_LOC * NT], F32)

            # PE warm-up: dummy matmuls keep TensorE busy (and un-throttle
            # the HAM clock gate) while the first input DMA lands.
            nc.vector.memset(warm_sb[:], 0)
            wps = px.tile([128, 1024], F32, name="ps")
            for _ in range(N_WARM):
                nc.tensor.matmul(wps[:, :512], lhsT=warm_sb[:, :128],
                                 rhs=warm_sb[:], start=True, stop=True)

            # Input DMA order is the critical path: the load phase is
            # HBM-bandwidth-bound (~330 GB/s aggregate), so order transfers
            # by when compute first needs them.  k-tile 6 holds only 16
            # valid rows (784 = 6*128 + 16) — transfer just those
            # partitions, and put it first so the layer-0 accumulation can
            # OPEN with it and close on the last full chunk.
            nc.sync.dma_start(out=aw_sb[:, 0:2 * AW_K],
                              in_=aw_d[:, 0:2 * AW_K])
            nc.sync.dma_start(out=aw_sb[0:16, K6:K6 + AW_K],
                              in_=aw_d[0:16, K6:K6 + AW_K])
            for k0, k1 in AW_CHUNKS[1:]:
                nc.sync.dma_start(
                    out=aw_sb[:, k0 * AW_K:k1 * AW_K],
                    in_=aw_d[:, k0 * AW_K:k1 * AW_K])
            # replica 0's weights right after the aw chunks (its layer 1
            # starts ~3us before any other replica's), then singles/pairs
            # in consumption order.
            RW = KH_T * D_H
            W1_CHUNKS = [(0, 1), (1, 2), (2, 4), (4, 6), (6, 8)]
            w1_sb = [(c0, w1p.tile([128, (c1 - c0) * RW], FP8,
                                   name=f"w1c{ci}"))
                     for ci, (c0, c1) in enumerate(W1_CHUNKS)]
            nc.sync.dma_start(out=w1_sb[0][1][:], in_=w1_d[:, 0:RW])
            nc.sync.dma_start(out=w2_sb[:], in_=w2_d[:])
            for ci in range(1, 5):
                c0, c1 = W1_CHUNKS[ci]
                nc.sync.dma_start(out=w1_sb[ci][1][:],
                                  in_=w1_d[:, c0 * RW:c1 * RW])

            # ---- layer 0: X^T = sigmoid(W0^T A^T), fp8 DoubleRow ----
            # The aw pack stores k-tile PAIR blocks [AT_k|AT_k+1|W0_k|
            # W0_k+1] so both DoubleRow operands have a contiguous 512 B
            # k-plane stride — DoubleRow only streams 2 fp8/cycle (216 ns
            # vs 427 ns per mm) when the two k-planes are adjacent in the
            # partition line.  k-outer within each m-pair so early chunks
            # are consumed while later chunks are still in flight.
            for mp in range(2):           # m pairs: (0,1), (2,3)
                ps = px.tile([128, 1024], F32, name="ps")
                # the 16-row k-tile 6 opens the accumulation (its DMA is
                # tiny and early), so each m-pair closes as soon as the
                # LAST full chunk lands — the sigmoid isn't pushed behind
                # a straggler tile.
                for m2 in range(2):
                    m = 2 * mp + m2
                    nc.tensor.matmul(
                        ps[:, m2 * NT:(m2 + 1) * NT],
                        lhsT=aw_sb[0:16, K6 + NT + m * 128:
                                   K6 + NT + (m + 1) * 128],
                        rhs=aw_sb[0:16, K6:K6 + NT],
                        start=True, stop=False,
                    )
                for j in range(3):        # k-tile pairs (0,1), (2,3), (4,5)
                    blk = j * 2 * AW_K
                    at2 = aw_sb[:, blk:blk + 2 * NT].rearrange(
                        "p (q n) -> p q n", q=2)
                    w02 = aw_sb[:, blk + 2 * NT:blk + 2 * AW_K].rearrange(
                        "p (q n) -> p q n", q=2)
                    for m2 in range(2):
                        m = 2 * mp + m2
                        nc.tensor.matmul(
                            ps[:, m2 * NT:(m2 + 1) * NT],
                            lhsT=w02[:, :, m * 128:(m + 1) * 128],
                            rhs=at2[:],
                            start=False, stop=(j == 2),
                            perf_mode=DR,
                        )
                # per-m-tile sigmoids: each [128,512] column range is its
                # own accumulation group, so sigmoid(m) fires one matmul
                # after the last aw chunk's semaphore instead of waiting
                # for the whole m-pair — the ACT stream starts ~1us
                # earlier and replica 0 unblocks sooner.
                for m2 in range(2):
                    m = 2 * mp + m2
                    nc.scalar.activation(
                        x_sb[:, m * NT:(m + 1) * NT],
                        ps[:, m2 * NT:(m2 + 1) * NT], SIG)

            # ---- per replica: layer 1 (fp8 DoubleRow), layer 2 deferred ----
            x3 = x_sb[:].rearrange("p (k n) -> p k n", k=KH_T)
            y_sbs = {}
            psz = {}

            def w1c3_of(r):
                for c0, w1c in reversed(w1_sb):
                    if r >= c0:
                        roff = (r - c0) * RW
                        return w1c[:, roff:roff + RW].rearrange(
                            "p (k n) -> p k n", k=KH_T)
                raise AssertionError(r)

            def l1_mm(ps, w13, mp, m2, kp):
                m = 2 * mp + m2
                nc.tensor.matmul(
                    ps[:, m2 * NT:(m2 + 1) * NT],
                    lhsT=w13[:, kp:kp + 2, m * 128:(m + 1) * 128],
                    rhs=x3[:, kp:kp + 2, :],
                    start=(kp == 0), stop=(kp == 2),
                    perf_mode=DR,
                )

            w23 = w2_sb[:].rearrange("p (k n) -> p k n", k=KH_T)

            def l2_pair(r, mp):
                # one DoubleRow matmul covers a k-tile pair; both pairs
                # accumulate into the same PSUM partials [0:16].
                y3r = y_sbs[r][:].rearrange("p (k n) -> p k n", k=KH_T)
                nc.tensor.matmul(
                    psz[r][0:W2C, :],
                    lhsT=w23[:, 2 * mp:2 * mp + 2, :],
                    rhs=y3r[:, 2 * mp:2 * mp + 2, :],
                    start=(mp == 0), stop=(mp == 1),
                    perf_mode=DR,
                )

            def l2_reduce(r):
                # single accumulation group -> one PSUM->SBUF copy
                nc.vector.tensor_copy(out=z_sb[:, r * NT:(r + 1) * NT],
                                      in_=psz[r][0:D_OUT, :])
                psz.pop(r)

            # replica 0: kp-outer across BOTH m-pair psums — its second
            # k-pair is gated on layer 0's second sigmoid, so consume kp0
            # for all four m-tiles first.
            w13 = w1c3_of(0)
            y_sbs[0] = yp.tile([128, KH_T * NT], FP8, name="y_sb")
            psz[0] = pz.tile([128, NT], F32, name="psz")
            ps_r0 = [px.tile([128, 1024], F32, name="ps") for _ in range(2)]
            for kp in (0, 2):
                for mp in range(2):
                    for m2 in range(2):
                        l1_mm(ps_r0[mp], w13, mp, m2, kp)
            for mp in range(2):
                nc.scalar.activation(
                    y_sbs[0][:, mp * 1024:(mp + 1) * 1024], ps_r0[mp][:], SIG)

            for r in range(1, R_LOC):
                w13 = w1c3_of(r)
                y_sbs[r] = yp.tile([128, KH_T * NT], FP8, name="y_sb")
                psz[r] = pz.tile([128, NT], F32, name="psz")
                last = (r == R_LOC - 1)
                for mp in range(2):
                    ps = px.tile([128, 1024], F32, name="ps")
                    for m2 in range(2):
                        for kp in (0, 2):
                            l1_mm(ps, w13, mp, m2, kp)
                    if last and mp == 1:
                        # last replica: split the final sigmoid so its
                        # layer 2 can start after the first half
                        nc.scalar.activation(
                            y_sbs[r][:, 1024:1536], ps[:, :512], SIG)
                        nc.scalar.activation(
                            y_sbs[r][:, 1536:2048], ps[:, 512:], SIG)
                    else:
                        nc.scalar.activation(
                            y_sbs[r][:, mp * 1024:(mp + 1) * 1024], ps[:], SIG)
                    if last:
                        if mp == 0:
                            # r6's layer 2 entirely here so its DVE reduce
                            # and the z[4:7] DMA clear out before the tail
                            l2_pair(r - 1, 0)
                            l2_pair(r - 1, 1)
                            l2_reduce(r - 1)
                            y_sbs.pop(r - 1)
                            nc.sync.dma_start(
                                out=zt_d[:, 4 * NT:7 * NT],
                                in_=z_sb[:, 4 * NT:7 * NT])
                    else:
                        # split-slot injection: pair mp of the PREVIOUS
                        # replica's layer 2 goes in slot mp, one sigmoid
                        # behind — its y-halves are guaranteed ready
                        l2_pair(r - 1, mp)
                        if mp == 1:
                            l2_reduce(r - 1)
                            y_sbs.pop(r - 1)
                            if r == 4:
                                nc.sync.dma_start(
                                    out=zt_d[:, :4 * NT], in_=z_sb[:, :4 * NT])

            # last replica's layer 2: the (k0,k1) DoubleRow pair needs only
            # the mp0 sigmoid; k2/k3 run as normal matmuls so each can
            # start right after its half of the split mp1 sigmoid.
            r = R_LOC - 1
            nc.tensor.matmul(
                psz[r][0:W2C, :],
                lhsT=w23[:, 0:2, :],
                rhs=y_sbs[r][:].rearrange("p (k n) -> p k n",
                                          k=KH_T)[:, 0:2, :],
                start=True, stop=False, perf_mode=DR,
                skip_group_check=True,
            )
            for k in (2, 3):
                nc.tensor.matmul(
                    psz[r][0:D_OUT, :],
                    lhsT=w2_sb[:, k * W2C:k * W2C + D_OUT],
                    rhs=y_sbs[r][:, k * NT:(k + 1) * NT],
                    start=False, stop=(k == 3),
                    skip_group_check=True,
                )
            # single copy + single DMA: one HBM write receipt on the tail
            nc.vector.tensor_copy(out=z_sb[:, r * NT:(r + 1) * NT],
                                  in_=psz[r][0:D_OUT, :])
            nc.sync.dma_start(out=zt_d[:, 7 * NT:8 * NT],
                              in_=z_sb[:, 7 * NT:8 * NT])

    nc.compile()
    return nc


def _pack_kxm(w, ktiles):
    """[K, M] -> [128, ktiles*M] with pack[p, k*M + m] = w[k*128 + p, m]."""
    K, M = w.shape
    assert K == ktiles * 128
    return np.ascontiguousarray(
        w.reshape(ktiles, 128, M).transpose(1, 0, 2).reshape(128, ktiles * M)
    )


def kernel(batch, W0, W1, W2, noise1):
    global last_results
    batch = np.asarray(batch, dtype=np.float32)
    W0 = np.asarray(W0, dtype=np.float32)
    W1 = np.asarray(W1, dtype=np.float32)
    W2 = np.asarray(W2, dtype=np.float32)
    noise1 = np.asarray(noise1, dtype=np.float32)

    bf = ml_dtypes.bfloat16
    f8 = mybir.dt.np(FP8)

    A = batch.reshape(BT, D_IN)
    ATp = np.zeros((KA, BT), np.float32)
    ATp[:D_IN] = A.T
    at_full = ATp.reshape(KA_T, 128, BT)          # [k, p, n]

    W0p = np.zeros((KA, D_H), np.float32)
    W0p[:D_IN] = W0
    w0_full = W0p.reshape(KA_T, 128, D_H)         # [k, p, m]

    noise = noise1.copy()
    noise[0] = 0.0
    W1n = W1[None] + noise                        # [16, 512, 512] fp32

    W2p = np.zeros((D_H, W2C), np.float32)
    W2p[:, :D_OUT] = W2
    w2_pack = _pack_kxm(W2p, KH_T).astype(f8)

    # per-replica-group W1 packs: [p, (r k n)]
    w1_packs = []
    for sg in range(SG):
        blk = W1n[sg * R_LOC:(sg + 1) * R_LOC]    # [8, 512, 512]
        p = blk.reshape(R_LOC, KH_T, 128, D_H).transpose(2, 0, 1, 3)
        w1_packs.append(np.ascontiguousarray(
            p.reshape(128, R_LOC * KH_T * D_H)).astype(f8))

    # per-token-group A^T|W0 packs in k-tile PAIR blocks:
    # [AT_2j | AT_2j+1 | W0_2j | W0_2j+1] x3, then [AT_6 | W0_6]
    aw_packs = []
    for tg in range(TG):
        at_sl = at_full[:, :, tg * NT:(tg + 1) * NT]      # [k, p, 512]
        blocks = []
        for j in range(3):
            blocks += [at_sl[2 * j], at_sl[2 * j + 1],
                       w0_full[2 * j], w0_full[2 * j + 1]]
        blocks += [at_sl[6], w0_full[6]]
        aw_packs.append(np.ascontiguousarray(
            np.concatenate(blocks, axis=1)).astype(f8))

    in_maps = []
    for c in range(N_CORES):
        sg, tg = c // TG, c % TG
        in_maps.append({
            "aw_pack": aw_packs[tg],
            "w1_pack": w1_packs[sg],
            "w2_pack": w2_pack,
        })

    if "nc" not in _CACHE:
        _CACHE["nc"] = _build_program()
    nc = _CACHE["nc"]

    trace = bool(int(os.environ.get("KERNEL_TRACE", "0")))
    res = run_bass_kernel_spmd(
        nc, in_maps, core_ids=list(range(N_CORES)), trace=trace)
    last_results = res

    out = np.empty((S, BT, D_OUT), np.float32)
    for c in range(N_CORES):
        sg, tg = c // TG, c % TG
        zt = np.asarray(res.results[c]["zt"], dtype=np.float32)  # [10, 8*512]
        for i in range(R_LOC):
            logits = zt[:, i * NT:(i + 1) * NT].T                # [512, 10]
            out[sg * R_LOC + i, tg * NT:(tg + 1) * NT] = (
                1.0 / (1.0 + np.exp(-logits)))
    return out.reshape(S, 32, 64, D_OUT)


# revision 34
# speedup vs baseline: 1.0587x; 1.0587x over previous
"""Trainium2 Bass kernel for the Noisy-Weights BNN MLP.

Computation (full problem):
  noise1[0] = 0;  W1n = W1[None] + noise1            # [16, 512, 512]
  X = sigmoid(A @ W0)        A = batch.reshape(2048, 784)
  Y_s = sigmoid(X @ W1n[s])
  Z_s = sigmoid(Y_s @ W2)    -> out [16, 32, 64, 10]

Sharding over 8 NeuronCores: 2 replica-groups (8 replicas each) x
4 token-groups (512 tokens each).  Each core redundantly computes the
shared layer 0 for its 512 tokens, then its 8 replicas of layers 1+2.

On-device layout: every matmul is a native out = lhsT.T @ rhs with the
contraction dim on SBUF partitions:
  layer0: lhsT = W0 tile, rhs = A^T tile -> psum X^T, sigmoid -> fp8
  layer1: lhsT = W1n tile, rhs = X^T     -> psum Y^T, sigmoid -> fp8
  layer2: lhsT = W2 tile [128k, 16pad], rhs = Y^T, two DoubleRow
          k-pair matmuls -> psum Z^T logits, one DVE copy to SBUF
Host applies the final sigmoid + transpose (tiny: 1.3 MB total).

Precision: all three layers run fp8e4m3 with DoubleRow perf mode
(2 k-tiles per matmul, 2x PE throughput when warm, half the DMA
bytes).  numpy-model rel-L2 vs the fp32 reference: 1.0e-2 (gate is
2e-2).  fp32 PSUM accumulation throughout.

Schedule notes: a few N=512 dummy matmuls warm the PE clock (HAM)
while the first DMA chunk lands; layer-0 A^T/W0 are packed in k-tile
pair blocks and DMA'd in chunks so compute starts after the first
256 KB; each replica's layer-2 k-pairs are issued one sigmoid behind
(pair mp in slot mp of the NEXT replica) so the PE FIFO never stalls
behind a not-yet-ready activation.  Steady state runs both TensorE
and ScalarE at ~100% occupancy (~2.0 us per replica).
"""

import os
import sys

import numpy as np
import ml_dtypes

if "/opt/trn_rl_repo" not in sys.path:
    sys.path.insert(0, "/opt/trn_rl_repo")

import concourse.bass as bass  # noqa: E402
import concourse.tile as tile  # noqa: E402
from concourse import bacc, mybir  # noqa: E402
from concourse.bass_utils import run_bass_kernel_spmd  # noqa: E402

# ---- problem constants (hardcoded; kernel.py must be self-contained) ----
S = 16           # noisy-weight replicas
BT = 2048        # batch tokens = 32 * 64
D_IN = 784
D_H = 512
D_OUT = 10
KA = 896         # 784 zero-padded to 7 * 128
N_CORES = 8
SG = 2           # replica groups
TG = 4           # token groups
R_LOC = S // SG          # replicas per core = 8
NT = BT // TG            # tokens per core = 512
KA_T = KA // 128         # 7 k-tiles for layer 0
KH_T = D_H // 128        # 4 k-tiles / m-tiles for hidden dims
AW_K = NT + D_H          # interleaved A^T|W0 stride per k-tile = 1024
W2C = 16                 # W2 k-tile columns: 10 outputs padded to 16 so the
                         # DoubleRow weight AP stride is 16 B-aligned

BF16 = mybir.dt.bfloat16
FP8 = mybir.dt.float8e4
F32 = mybir.dt.float32
DR = mybir.MatmulPerfMode.DoubleRow

# Dummy matmuls covering the first input-DMA wait (~7.5us -> ~10.4us: DMA
# can't start before the NEFF preamble ends and its completion semaphore
# takes ~0.75us after the data lands).  Keeping the PE busy the whole time
# both avoids the idle gap and lets the HAM clock gate reach 2.4 GHz
# before the first real matmul — cold DR matmuls are 2x slower.  N=512
# streaming matmuls (~85% duty) are needed to register as "busy" with
# the HAM activity window; short N=128 ones (~50% duty) leave it cold.
N_WARM = 8

_CACHE = {}

last_results = None  # BassKernelResults of the most recent run (for test.py)


def _build_program():
    """One SPMD Bass program; per-core differences live entirely in data."""
    nc = bacc.Bacc(None, target_bir_lowering=False, debug=False,
                   enable_partition_id=False)

    # layer-0 inputs interleaved per k-tile: aw[:, k*1024+0:512] = A^T k-tile,
    # aw[:, k*1024+512:1024] = W0 k-tile
    aw_d = nc.dram_tensor("aw_pack", [128, KA_T * AW_K], FP8,
                          kind="ExternalInput")
    w1_d = nc.dram_tensor("w1_pack", [128, R_LOC * KH_T * D_H], FP8,
                          kind="ExternalInput")
    w2_d = nc.dram_tensor("w2_pack", [128, KH_T * W2C], FP8,
                          kind="ExternalInput")
    zt_d = nc.dram_tensor("zt", [D_OUT, R_LOC * NT], F32, kind="ExternalOutput")

    SIG = mybir.ActivationFunctionType.Sigmoid
    AW_CHUNKS = [(0, 2), (2, 4), (4, 6)]   # full k-tile ranges per chunk
    K6 = (KA_T - 1) * AW_K                 # col offset of the 16-row k-tile 6

    with tile.TileContext(nc) as tc:
        with (
            tc.tile_pool(name="consts", bufs=1) as consts,
            tc.tile_pool(name="w1p", bufs=1) as w1p,
            tc.tile_pool(name="yp", bufs=3) as yp,
            tc.tile_pool(name="px", bufs=3, space="PSUM") as px,
            tc.tile_pool(name="pz", bufs=2, space="PSUM") as pz,
        ):
            warm_sb = consts.tile([128, 512], BF16)
            aw_sb = consts.tile([128, KA_T * AW_K], FP8)
            w2_sb = consts.tile([128, KH_T * W2C], FP8)
            x_sb = consts.tile([128, KH_T * NT], FP8)
            z_sb = consts.tile([D_OUT, R_LOC * NT], F32)

            # PE warm-up: dummy matmuls keep TensorE busy (and un-throttle
            # the HAM clock gate) while the first input DMA lands.
            nc.vector.memset(warm_sb[:], 0)
            wps = px.tile([128, 1024], F32, name="ps")
            for _ in range(N_WARM):
                nc.tensor.matmul(wps[:, :512], lhsT=warm_sb[:, :128],
                                 rhs=warm_sb[:], start=True, stop=True)

            # Input DMA order is the critical path: the load phase is
            # HBM-bandwidth-bound (~330 GB/s aggregate), so order transfers
            # by when compute first needs them.  k-tile 6 holds only 16
            # valid rows (784 = 6*128 + 16) — transfer just those
            # partitions, and put it first so the layer-0 accumulation can
            # OPEN with it and close on the last full chunk.
            nc.sync.dma_start(out=aw_sb[:, 0:2 * AW_K],
                              in_=aw_d[:, 0:2 * AW_K])
            nc.sync.dma_start(out=aw_sb[0:16, K6:K6 + AW_K],
                              in_=aw_d[0:16, K6:K6 + AW_K])
            for k0, k1 in AW_CHUNKS[1:]:
                nc.sync.dma_start(
                    out=aw_sb[:, k0 * AW_K:k1 * AW_K],
                    in_=aw_d[:, k0 * AW_K:k1 * AW_K])
            # replica 0's weights right after the aw chunks (its layer 1
            # starts ~3us before any other replica's), then singles/pairs
            # in consumption order.
            RW = KH_T * D_H
            W1_CHUNKS = [(0, 1), (1, 2), (2, 4), (4, 6), (6, 8)]
            w1_sb = [(c0, w1p.tile([128, (c1 - c0) * RW], FP8,
                                   name=f"w1c{ci}"))
                     for ci, (c0, c1) in enumerate(W1_CHUNKS)]
            nc.sync.dma_start(out=w1_sb[0][1][:], in_=w1_d[:, 0:RW])
            nc.sync.dma_start(out=w2_sb[:], in_=w2_d[:])
            for ci in range(1, 5):
                c0, c1 = W1_CHUNKS[ci]
                nc.sync.dma_start(out=w1_sb[ci][1][:],
                                  in_=w1_d[:, c0 * RW:c1 * RW])

            # ---- layer 0: X^T = sigmoid(W0^T A^T), fp8 DoubleRow ----
            # The aw pack stores k-tile PAIR blocks [AT_k|AT_k+1|W0_k|
            # W0_k+1] so both DoubleRow operands have a contiguous 512 B
            # k-plane stride — DoubleRow only streams 2 fp8/cycle (216 ns
            # vs 427 ns per mm) when the two k-planes are adjacent in the
            # partition line.  k-outer within each m-pair so early chunks
            # are consumed while later chunks are still in flight.
            for mp in range(2):           # m pairs: (0,1), (2,3)
                ps = px.tile([128, 1024], F32, name="ps")
                # the 16-row k-tile 6 opens the accumulation (its DMA is
                # tiny and early), so each m-pair closes as soon as the
                # LAST full chunk lands — the sigmoid isn't pushed behind
                # a straggler tile.
                for m2 in range(2):
                    m = 2 * mp + m2
                    nc.tensor.matmul(
                        ps[:, m2 * NT:(m2 + 1) * NT],
                        lhsT=aw_sb[0:16, K6 + NT + m * 128:
                                   K6 + NT + (m + 1) * 128],
                        rhs=aw_sb[0:16, K6:K6 + NT],
                        start=True, stop=False,
                    )
                for j in range(3):        # k-tile pairs (0,1), (2,3), (4,5)
                    blk = j * 2 * AW_K
                    at2 = aw_sb[:, blk:blk + 2 * NT].rearrange(
                        "p (q n) -> p q n", q=2)
                    w02 = aw_sb[:, blk + 2 * NT:blk + 2 * AW_K].rearrange(
                        "p (q n) -> p q n", q=2)
                    for m2 in range(2):
                        m = 2 * mp + m2
                        nc.tensor.matmul(
                            ps[:, m2 * NT:(m2 + 1) * NT],
                            lhsT=w02[:, :, m * 128:(m + 1) * 128],
                            rhs=at2[:],
                            start=False, stop=(j == 2),
                            perf_mode=DR,
                        )
                # one sigmoid per m-pair: finer [128,512] splits measure
                # WORSE (45.5us vs 39.6) — the scheduler's counting-
                # semaphore waits get assigned to later PE positions and
                # delay the whole ACT stream.
                nc.scalar.activation(
                    x_sb[:, mp * 1024:(mp + 1) * 1024], ps[:], SIG)

            # ---- per replica: layer 1 (fp8 DoubleRow), layer 2 deferred ----
            x3 = x_sb[:].rearrange("p (k n) -> p k n", k=KH_T)
            y_sbs = {}
            psz = {}

            def w1c3_of(r):
                for c0, w1c in reversed(w1_sb):
                    if r >= c0:
                        roff = (r - c0) * RW
                        return w1c[:, roff:roff + RW].rearrange(
                            "p (k n) -> p k n", k=KH_T)
                raise AssertionError(r)

            def l1_mm(ps, w13, mp, m2, kp):
                m = 2 * mp + m2
                nc.tensor.matmul(
                    ps[:, m2 * NT:(m2 + 1) * NT],
                    lhsT=w13[:, kp:kp + 2, m * 128:(m + 1) * 128],
                    rhs=x3[:, kp:kp + 2, :],
                    start=(kp == 0), stop=(kp == 2),
                    perf_mode=DR,
                )

            w23 = w2_sb[:].rearrange("p (k n) -> p k n", k=KH_T)

            def l2_pair(r, mp):
                # one DoubleRow matmul covers a k-tile pair; both pairs
                # accumulate into the same PSUM partials [0:16].
                y3r = y_sbs[r][:].rearrange("p (k n) -> p k n", k=KH_T)
                nc.tensor.matmul(
                    psz[r][0:W2C, :],
                    lhsT=w23[:, 2 * mp:2 * mp + 2, :],
                    rhs=y3r[:, 2 * mp:2 * mp + 2, :],
                    start=(mp == 0), stop=(mp == 1),
                    perf_mode=DR,
                )

            def l2_reduce(r):
                # single accumulation group -> one PSUM->SBUF copy
                nc.vector.tensor_copy(out=z_sb[:, r * NT:(r + 1) * NT],
                                      in_=psz[r][0:D_OUT, :])
                psz.pop(r)

            # replica 0: kp-outer across BOTH m-pair psums — its second
            # k-pair is gated on layer 0's second sigmoid, so consume kp0
            # for all four m-tiles first.
            w13 = w1c3_of(0)
            y_sbs[0] = yp.tile([128, KH_T * NT], FP8, name="y_sb")
            psz[0] = pz.tile([128, NT], F32, name="psz")
            ps_r0 = [px.tile([128, 1024], F32, name="ps") for _ in range(2)]
            for kp in (0, 2):
                for mp in range(2):
                    for m2 in range(2):
                        l1_mm(ps_r0[mp], w13, mp, m2, kp)
            for mp in range(2):
                nc.scalar.activation(
                    y_sbs[0][:, mp * 1024:(mp + 1) * 1024], ps_r0[mp][:], SIG)

            for r in range(1, R_LOC):
                w13 = w1c3_of(r)
                y_sbs[r] = yp.tile([128, KH_T * NT], FP8, name="y_sb")
                psz[r] = pz.tile([128, NT], F32, name="psz")
                last = (r == R_LOC - 1)
                for mp in range(2):
                    ps = px.tile([128, 1024], F32, name="ps")
                    for m2 in range(2):
                        for kp in (0, 2):
                            l1_mm(ps, w13, mp, m2, kp)
                    if last and mp == 1:
                        # last replica: split the final sigmoid so its
                        # layer 2 can start after the first half
                        nc.scalar.activation(
                            y_sbs[r][:, 1024:1536], ps[:, :512], SIG)
                        nc.scalar.activation(
                            y_sbs[r][:, 1536:2048], ps[:, 512:], SIG)
                    else:
                        nc.scalar.activation(
                            y_sbs[r][:, mp * 1024:(mp + 1) * 1024], ps[:], SIG)
                    if last:
                        if mp == 0:
                            # r6's layer 2 entirely here so its DVE reduce
                            # and the z[4:7] DMA clear out before the tail
                            l2_pair(r - 1, 0)
                            l2_pair(r - 1, 1)
                            l2_reduce(r - 1)
                            y_sbs.pop(r - 1)
                            nc.sync.dma_start(
                                out=zt_d[:, 4 * NT:7 * NT],
                                in_=z_sb[:, 4 * NT:7 * NT])
                    else:
                        # split-slot injection: pair mp of the PREVIOUS
                        # replica's layer 2 goes in slot mp, one sigmoid
                        # behind — its y-halves are guaranteed ready
                        l2_pair(r - 1, mp)
                        if mp == 1:
                            l2_reduce(r - 1)
                            y_sbs.pop(r - 1)
                            if r == 4:
                                nc.sync.dma_start(
                                    out=zt_d[:, :4 * NT], in_=z_sb[:, :4 * NT])

            # last replica's layer 2: the (k0,k1) DoubleRow pair needs only
            # the mp0 sigmoid; k2/k3 run as normal matmuls so each can
            # start right after its half of the split mp1 sigmoid.
            r = R_LOC - 1
            nc.tensor.matmul(
                psz[r][0:W2C, :],
                lhsT=w23[:, 0:2, :],
                rhs=y_sbs[r][:].rearrange("p (k n) -> p k n",
                                          k=KH_T)[:, 0:2, :],
                start=True, stop=False, perf_mode=DR,
                skip_group_check=True,
            )
            for k in (2, 3):
                nc.tensor.matmul(
                    psz[r][0:D_OUT, :],
                    lhsT=w2_sb[:, k * W2C:k * W2C + D_OUT],
                    rhs=y_sbs[r][:, k * NT:(k + 1) * NT],
                    start=False, stop=(k == 3),
                    skip_group_check=True,
                )
            # single copy + single DMA: one HBM write receipt on the tail
            nc.vector.tensor_copy(out=z_sb[:, r * NT:(r + 1) * NT],
                                  in_=psz[r][0:D_OUT, :])
            nc.sync.dma_start(out=zt_d[:, 7 * NT:8 * NT],
                              in_=z_sb[:, 7 * NT:8 * NT])

    nc.compile()
    return nc


def _pack_kxm(w, ktiles):
    """[K, M] -> [128, ktiles*M] with pack[p, k*M + m] = w[k*128 + p, m]."""
    K, M = w.shape
    assert K == ktiles * 128
    return np.ascontiguousarray(
        w.reshape(ktiles, 128, M).transpose(1, 0, 2).reshape(128, ktiles * M)
    )


def kernel(batch, W0, W1, W2, noise1):
    global last_results
    batch = np.asarray(batch, dtype=np.float32)
    W0 = np.asarray(W0, dtype=np.float32)
    W1 = np.asarray(W1, dtype=np.float32)
    W2 = np.asarray(W2, dtype=np.float32)
    noise1 = np.asarray(noise1, dtype=np.float32)

    bf = ml_dtypes.bfloat16
    f8 = mybir.dt.np(FP8)

    A = batch.reshape(BT, D_IN)
    ATp = np.zeros((KA, BT), np.float32)
    ATp[:D_IN] = A.T
    at_full = ATp.reshape(KA_T, 128, BT)          # [k, p, n]

    W0p = np.zeros((KA, D_H), np.float32)
    W0p[:D_IN] = W0
    w0_full = W0p.reshape(KA_T, 128, D_H)         # [k, p, m]

    noise = noise1.copy()
    noise[0] = 0.0
    W1n = W1[None] + noise                        # [16, 512, 512] fp32

    W2p = np.zeros((D_H, W2C), np.float32)
    W2p[:, :D_OUT] = W2
    w2_pack = _pack_kxm(W2p, KH_T).astype(f8)

    # per-replica-group W1 packs: [p, (r k n)]
    w1_packs = []
    for sg in range(SG):
        blk = W1n[sg * R_LOC:(sg + 1) * R_LOC]    # [8, 512, 512]
        p = blk.reshape(R_LOC, KH_T, 128, D_H).transpose(2, 0, 1, 3)
        w1_packs.append(np.ascontiguousarray(
            p.reshape(128, R_LOC * KH_T * D_H)).astype(f8))

    # per-token-group A^T|W0 packs in k-tile PAIR blocks:
    # [AT_2j | AT_2j+1 | W0_2j | W0_2j+1] x3, then [AT_6 | W0_6]
    aw_packs = []
    for tg in range(TG):
        at_sl = at_full[:, :, tg * NT:(tg + 1) * NT]      # [k, p, 512]
        blocks = []
        for j in range(3):
            blocks += [at_sl[2 * j], at_sl[2 * j + 1],
                       w0_full[2 * j], w0_full[2 * j + 1]]
        blocks += [at_sl[6], w0_full[6]]
        aw_packs.append(np.ascontiguousarray(
            np.concatenate(blocks, axis=1)).astype(f8))

    in_maps = []
    for c in range(N_CORES):
        sg, tg = c // TG, c % TG
        in_maps.append({
            "aw_pack": aw_packs[tg],
            "w1_pack": w1_packs[sg],
            "w2_pack": w2_pack,
        })

    if "nc" not in _CACHE:
        _CACHE["nc"] = _build_program()
    nc = _CACHE["nc"]

    trace = bool(int(os.environ.get("KERNEL_TRACE", "0")))
    res = run_bass_kernel_spmd(
        nc, in_maps, core_ids=list(range(N_CORES)), trace=trace)
    last_results = res

    out = np.empty((S, BT, D_OUT), np.float32)
    for c in range(N_CORES):
        sg, tg = c // TG, c % TG
        zt = np.asarray(res.results[c]["zt"], dtype=np.float32)  # [10, 8*512]
        for i in range(R_LOC):
            logits = zt[:, i * NT:(i + 1) * NT].T                # [512, 10]
            out[sg * R_LOC + i, tg * NT:(tg + 1) * NT] = (
                1.0 / (1.0 + np.exp(-logits)))
    return out.reshape(S, 32, 64, D_OUT)


# revision 35
# speedup vs baseline: 1.1039x; 1.0427x over previous
"""Trainium2 Bass kernel for the Noisy-Weights BNN MLP.

Computation (full problem):
  noise1[0] = 0;  W1n = W1[None] + noise1            # [16, 512, 512]
  X = sigmoid(A @ W0)        A = batch.reshape(2048, 784)
  Y_s = sigmoid(X @ W1n[s])
  Z_s = sigmoid(Y_s @ W2)    -> out [16, 32, 64, 10]

Sharding over 8 NeuronCores: 2 replica-groups (8 replicas each) x
4 token-groups (512 tokens each).  Each core redundantly computes the
shared layer 0 for its 512 tokens, then its 8 replicas of layers 1+2.

On-device layout: every matmul is a native out = lhsT.T @ rhs with the
contraction dim on SBUF partitions:
  layer0: lhsT = W0 tile, rhs = A^T tile -> psum X^T, sigmoid -> fp8
  layer1: lhsT = W1n tile, rhs = X^T     -> psum Y^T, sigmoid -> fp8
  layer2: lhsT = W2 tile [128k, 16pad], rhs = Y^T, two DoubleRow
          k-pair matmuls -> psum Z^T logits, one DVE copy to SBUF
Host applies the final sigmoid + transpose (tiny: 1.3 MB total).

Precision: all three layers run fp8e4m3 with DoubleRow perf mode
(2 k-tiles per matmul, 2x PE throughput when warm, half the DMA
bytes).  numpy-model rel-L2 vs the fp32 reference: 1.0e-2 (gate is
2e-2).  fp32 PSUM accumulation throughout.

Schedule notes: a few N=512 dummy matmuls warm the PE clock (HAM)
while the first DMA chunk lands; layer-0 A^T/W0 are packed in k-tile
pair blocks and DMA'd in chunks so compute starts after the first
256 KB; each replica's layer-2 k-pairs are issued one sigmoid behind
(pair mp in slot mp of the NEXT replica) so the PE FIFO never stalls
behind a not-yet-ready activation.  Steady state runs both TensorE
and ScalarE at ~100% occupancy (~2.0 us per replica).
"""

import os
import sys

import numpy as np
import ml_dtypes

if "/opt/trn_rl_repo" not in sys.path:
    sys.path.insert(0, "/opt/trn_rl_repo")

import concourse.bass as bass  # noqa: E402
import concourse.tile as tile  # noqa: E402
from concourse import bacc, mybir  # noqa: E402
from concourse.bass_utils import run_bass_kernel_spmd  # noqa: E402

# ---- problem constants (hardcoded; kernel.py must be self-contained) ----
S = 16           # noisy-weight replicas
BT = 2048        # batch tokens = 32 * 64
D_IN = 784
D_H = 512
D_OUT = 10
KA = 896         # 784 zero-padded to 7 * 128
N_CORES = 8
SG = 2           # replica groups
TG = 4           # token groups
R_LOC = S // SG          # replicas per core = 8
NT = BT // TG            # tokens per core = 512
KA_T = KA // 128         # 7 k-tiles for layer 0
KH_T = D_H // 128        # 4 k-tiles / m-tiles for hidden dims
AW_K = NT + D_H          # interleaved A^T|W0 stride per k-tile = 1024
W2C = 16                 # W2 k-tile columns: 10 outputs padded to 16 so the
                         # DoubleRow weight AP stride is 16 B-aligned

BF16 = mybir.dt.bfloat16
FP8 = mybir.dt.float8e4
F32 = mybir.dt.float32
DR = mybir.MatmulPerfMode.DoubleRow

# Dummy matmuls covering the first input-DMA wait (~7.5us -> ~10.4us: DMA
# can't start before the NEFF preamble ends and its completion semaphore
# takes ~0.75us after the data lands).  Keeping the PE busy the whole time
# both avoids the idle gap and lets the HAM clock gate reach 2.4 GHz
# before the first real matmul — cold DR matmuls are 2x slower.  N=512
# streaming matmuls (~85% duty) are needed to register as "busy" with
# the HAM activity window; short N=128 ones (~50% duty) leave it cold.
N_WARM = 11

_CACHE = {}

last_results = None  # BassKernelResults of the most recent run (for test.py)


def _build_program():
    """One SPMD Bass program; per-core differences live entirely in data."""
    nc = bacc.Bacc(None, target_bir_lowering=False, debug=False,
                   enable_partition_id=False)

    # layer-0 inputs interleaved per k-tile: aw[:, k*1024+0:512] = A^T k-tile,
    # aw[:, k*1024+512:1024] = W0 k-tile
    aw_d = nc.dram_tensor("aw_pack", [128, KA_T * AW_K], FP8,
                          kind="ExternalInput")
    w1_d = nc.dram_tensor("w1_pack", [128, R_LOC * KH_T * D_H], FP8,
                          kind="ExternalInput")
    w2_d = nc.dram_tensor("w2_pack", [128, KH_T * W2C], FP8,
                          kind="ExternalInput")
    zt_d = nc.dram_tensor("zt", [D_OUT, R_LOC * NT], F32, kind="ExternalOutput")

    SIG = mybir.ActivationFunctionType.Sigmoid
    AW_CHUNKS = [(0, 2), (2, 4), (4, 6)]   # full k-tile ranges per chunk
    K6 = (KA_T - 1) * AW_K                 # col offset of the 16-row k-tile 6

    with tile.TileContext(nc) as tc:
        with (
            tc.tile_pool(name="consts", bufs=1) as consts,
            tc.tile_pool(name="w1p", bufs=1) as w1p,
            tc.tile_pool(name="yp", bufs=3) as yp,
            tc.tile_pool(name="px", bufs=3, space="PSUM") as px,
            tc.tile_pool(name="pz", bufs=2, space="PSUM") as pz,
        ):
            warm_sb = consts.tile([128, 512], BF16)
            aw_sb = consts.tile([128, KA_T * AW_K], FP8)
            w2_sb = consts.tile([128, KH_T * W2C], FP8)
            x_sb = consts.tile([128, KH_T * NT], FP8)
            z_sb = consts.tile([D_OUT, R_LOC * NT], F32)

            # PE warm-up: dummy matmuls keep TensorE busy (and un-throttle
            # the HAM clock gate) while the first input DMA lands.
            nc.vector.memset(warm_sb[:], 0)
            wps = px.tile([128, 1024], F32, name="ps")
            for _ in range(N_WARM):
                nc.tensor.matmul(wps[:, :512], lhsT=warm_sb[:, :128],
                                 rhs=warm_sb[:], start=True, stop=True)

            # Input DMA order is the critical path: the load phase is
            # HBM-bandwidth-bound (~330 GB/s aggregate), so order transfers
            # by when compute first needs them.  k-tile 6 holds only 16
            # valid rows (784 = 6*128 + 16) — transfer just those
            # partitions, and put it first so the layer-0 accumulation can
            # OPEN with it and close on the last full chunk.
            nc.sync.dma_start(out=aw_sb[:, 0:2 * AW_K],
                              in_=aw_d[:, 0:2 * AW_K])
            nc.sync.dma_start(out=aw_sb[0:16, K6:K6 + AW_K],
                              in_=aw_d[0:16, K6:K6 + AW_K])
            for k0, k1 in AW_CHUNKS[1:]:
                nc.sync.dma_start(
                    out=aw_sb[:, k0 * AW_K:k1 * AW_K],
                    in_=aw_d[:, k0 * AW_K:k1 * AW_K])
            # replica 0's weights right after the aw chunks (its layer 1
            # starts ~3us before any other replica's), then singles/pairs
            # in consumption order.
            RW = KH_T * D_H
            W1_CHUNKS = [(0, 1), (1, 2), (2, 4), (4, 6), (6, 8)]
            w1_sb = [(c0, w1p.tile([128, (c1 - c0) * RW], FP8,
                                   name=f"w1c{ci}"))
                     for ci, (c0, c1) in enumerate(W1_CHUNKS)]
            nc.sync.dma_start(out=w1_sb[0][1][:], in_=w1_d[:, 0:RW])
            nc.sync.dma_start(out=w2_sb[:], in_=w2_d[:])
            for ci in range(1, 5):
                c0, c1 = W1_CHUNKS[ci]
                nc.sync.dma_start(out=w1_sb[ci][1][:],
                                  in_=w1_d[:, c0 * RW:c1 * RW])

            # ---- layer 0: X^T = sigmoid(W0^T A^T), fp8 DoubleRow ----
            # The aw pack stores k-tile PAIR blocks [AT_k|AT_k+1|W0_k|
            # W0_k+1] so both DoubleRow operands have a contiguous 512 B
            # k-plane stride — DoubleRow only streams 2 fp8/cycle (216 ns
            # vs 427 ns per mm) when the two k-planes are adjacent in the
            # partition line.  k-outer within each m-pair so early chunks
            # are consumed while later chunks are still in flight.
            for mp in range(2):           # m pairs: (0,1), (2,3)
                ps = px.tile([128, 1024], F32, name="ps")
                # the 16-row k-tile 6 opens the accumulation (its DMA is
                # tiny and early), so each m-pair closes as soon as the
                # LAST full chunk lands — the sigmoid isn't pushed behind
                # a straggler tile.
                for m2 in range(2):
                    m = 2 * mp + m2
                    nc.tensor.matmul(
                        ps[:, m2 * NT:(m2 + 1) * NT],
                        lhsT=aw_sb[0:16, K6 + NT + m * 128:
                                   K6 + NT + (m + 1) * 128],
                        rhs=aw_sb[0:16, K6:K6 + NT],
                        start=True, stop=False,
                    )
                for j in range(3):        # k-tile pairs (0,1), (2,3), (4,5)
                    blk = j * 2 * AW_K
                    at2 = aw_sb[:, blk:blk + 2 * NT].rearrange(
                        "p (q n) -> p q n", q=2)
                    w02 = aw_sb[:, blk + 2 * NT:blk + 2 * AW_K].rearrange(
                        "p (q n) -> p q n", q=2)
                    for m2 in range(2):
                        m = 2 * mp + m2
                        nc.tensor.matmul(
                            ps[:, m2 * NT:(m2 + 1) * NT],
                            lhsT=w02[:, :, m * 128:(m + 1) * 128],
                            rhs=at2[:],
                            start=False, stop=(j == 2),
                            perf_mode=DR,
                        )
                # one sigmoid per m-pair: finer [128,512] splits measure
                # WORSE (45.5us vs 39.6) — the scheduler's counting-
                # semaphore waits get assigned to later PE positions and
                # delay the whole ACT stream.
                nc.scalar.activation(
                    x_sb[:, mp * 1024:(mp + 1) * 1024], ps[:], SIG)

            # ---- per replica: layer 1 (fp8 DoubleRow), layer 2 deferred ----
            x3 = x_sb[:].rearrange("p (k n) -> p k n", k=KH_T)
            y_sbs = {}
            psz = {}

            def w1c3_of(r):
                for c0, w1c in reversed(w1_sb):
                    if r >= c0:
                        roff = (r - c0) * RW
                        return w1c[:, roff:roff + RW].rearrange(
                            "p (k n) -> p k n", k=KH_T)
                raise AssertionError(r)

            def l1_mm(ps, w13, mp, m2, kp):
                m = 2 * mp + m2
                nc.tensor.matmul(
                    ps[:, m2 * NT:(m2 + 1) * NT],
                    lhsT=w13[:, kp:kp + 2, m * 128:(m + 1) * 128],
                    rhs=x3[:, kp:kp + 2, :],
                    start=(kp == 0), stop=(kp == 2),
                    perf_mode=DR,
                )

            w23 = w2_sb[:].rearrange("p (k n) -> p k n", k=KH_T)

            def l2_pair(r, mp):
                # one DoubleRow matmul covers a k-tile pair; both pairs
                # accumulate into the same PSUM partials [0:16].
                y3r = y_sbs[r][:].rearrange("p (k n) -> p k n", k=KH_T)
                nc.tensor.matmul(
                    psz[r][0:W2C, :],
                    lhsT=w23[:, 2 * mp:2 * mp + 2, :],
                    rhs=y3r[:, 2 * mp:2 * mp + 2, :],
                    start=(mp == 0), stop=(mp == 1),
                    perf_mode=DR,
                )

            def l2_reduce(r):
                # single accumulation group -> one PSUM->SBUF copy
                nc.vector.tensor_copy(out=z_sb[:, r * NT:(r + 1) * NT],
                                      in_=psz[r][0:D_OUT, :])
                psz.pop(r)

            # replica 0: kp-outer across BOTH m-pair psums — its second
            # k-pair is gated on layer 0's second sigmoid, so consume kp0
            # for all four m-tiles first.
            w13 = w1c3_of(0)
            y_sbs[0] = yp.tile([128, KH_T * NT], FP8, name="y_sb")
            psz[0] = pz.tile([128, NT], F32, name="psz")
            ps_r0 = [px.tile([128, 1024], F32, name="ps") for _ in range(2)]
            for kp in (0, 2):
                for mp in range(2):
                    for m2 in range(2):
                        l1_mm(ps_r0[mp], w13, mp, m2, kp)
            for mp in range(2):
                nc.scalar.activation(
                    y_sbs[0][:, mp * 1024:(mp + 1) * 1024], ps_r0[mp][:], SIG)

            for r in range(1, R_LOC):
                w13 = w1c3_of(r)
                y_sbs[r] = yp.tile([128, KH_T * NT], FP8, name="y_sb")
                psz[r] = pz.tile([128, NT], F32, name="psz")
                last = (r == R_LOC - 1)
                for mp in range(2):
                    ps = px.tile([128, 1024], F32, name="ps")
                    for m2 in range(2):
                        for kp in (0, 2):
                            l1_mm(ps, w13, mp, m2, kp)
                    if last and mp == 1:
                        # last replica: split the final sigmoid so its
                        # layer 2 can start after the first half
                        nc.scalar.activation(
                            y_sbs[r][:, 1024:1536], ps[:, :512], SIG)
                        nc.scalar.activation(
                            y_sbs[r][:, 1536:2048], ps[:, 512:], SIG)
                    else:
                        nc.scalar.activation(
                            y_sbs[r][:, mp * 1024:(mp + 1) * 1024], ps[:], SIG)
                    if last:
                        if mp == 0:
                            # r6's layer 2 entirely here so its DVE reduce
                            # and the z[4:7] DMA clear out before the tail
                            l2_pair(r - 1, 0)
                            l2_pair(r - 1, 1)
                            l2_reduce(r - 1)
                            y_sbs.pop(r - 1)
                            nc.sync.dma_start(
                                out=zt_d[:, 4 * NT:7 * NT],
                                in_=z_sb[:, 4 * NT:7 * NT])
                    else:
                        # split-slot injection: pair mp of the PREVIOUS
                        # replica's layer 2 goes in slot mp, one sigmoid
                        # behind — its y-halves are guaranteed ready
                        l2_pair(r - 1, mp)
                        if mp == 1:
                            l2_reduce(r - 1)
                            y_sbs.pop(r - 1)
                            if r == 4:
                                nc.sync.dma_start(
                                    out=zt_d[:, :4 * NT], in_=z_sb[:, :4 * NT])

            # last replica's layer 2: the (k0,k1) DoubleRow pair needs only
            # the mp0 sigmoid; k2/k3 run as normal matmuls so each can
            # start right after its half of the split mp1 sigmoid.
            r = R_LOC - 1
            nc.tensor.matmul(
                psz[r][0:W2C, :],
                lhsT=w23[:, 0:2, :],
                rhs=y_sbs[r][:].rearrange("p (k n) -> p k n",
                                          k=KH_T)[:, 0:2, :],
                start=True, stop=False, perf_mode=DR,
                skip_group_check=True,
            )
            for k in (2, 3):
                nc.tensor.matmul(
                    psz[r][0:D_OUT, :],
                    lhsT=w2_sb[:, k * W2C:k * W2C + D_OUT],
                    rhs=y_sbs[r][:, k * NT:(k + 1) * NT],
                    start=False, stop=(k == 3),
                    skip_group_check=True,
                )
            # single copy + single DMA: one HBM write receipt on the tail
            nc.vector.tensor_copy(out=z_sb[:, r * NT:(r + 1) * NT],
                                  in_=psz[r][0:D_OUT, :])
            nc.sync.dma_start(out=zt_d[:, 7 * NT:8 * NT],
                              in_=z_sb[:, 7 * NT:8 * NT])

    nc.compile()
    return nc


def _pack_kxm(w, ktiles):
    """[K, M] -> [128, ktiles*M] with pack[p, k*M + m] = w[k*128 + p, m]."""
    K, M = w.shape
    assert K == ktiles * 128
    return np.ascontiguousarray(
        w.reshape(ktiles, 128, M).transpose(1, 0, 2).reshape(128, ktiles * M)
    )


def kernel(batch, W0, W1, W2, noise1):
    global last_results
    batch = np.asarray(batch, dtype=np.float32)
    W0 = np.asarray(W0, dtype=np.float32)
    W1 = np.asarray(W1, dtype=np.float32)
    W2 = np.asarray(W2, dtype=np.float32)
    noise1 = np.asarray(noise1, dtype=np.float32)

    bf = ml_dtypes.bfloat16
    f8 = mybir.dt.np(FP8)

    A = batch.reshape(BT, D_IN)
    ATp = np.zeros((KA, BT), np.float32)
    ATp[:D_IN] = A.T
    at_full = ATp.reshape(KA_T, 128, BT)          # [k, p, n]

    W0p = np.zeros((KA, D_H), np.float32)
    W0p[:D_IN] = W0
    w0_full = W0p.reshape(KA_T, 128, D_H)         # [k, p, m]

    noise = noise1.copy()
    noise[0] = 0.0
    W1n = W1[None] + noise                        # [16, 512, 512] fp32

    W2p = np.zeros((D_H, W2C), np.float32)
    W2p[:, :D_OUT] = W2
    w2_pack = _pack_kxm(W2p, KH_T).astype(f8)

    # per-replica-group W1 packs: [p, (r k n)]
    w1_packs = []
    for sg in range(SG):
        blk = W1n[sg * R_LOC:(sg + 1) * R_LOC]    # [8, 512, 512]
        p = blk.reshape(R_LOC, KH_T, 128, D_H).transpose(2, 0, 1, 3)
        w1_packs.append(np.ascontiguousarray(
            p.reshape(128, R_LOC * KH_T * D_H)).astype(f8))

    # per-token-group A^T|W0 packs in k-tile PAIR blocks:
    # [AT_2j | AT_2j+1 | W0_2j | W0_2j+1] x3, then [AT_6 | W0_6]
    aw_packs = []
    for tg in range(TG):
        at_sl = at_full[:, :, tg * NT:(tg + 1) * NT]      # [k, p, 512]
        blocks = []
        for j in range(3):
            blocks += [at_sl[2 * j], at_sl[2 * j + 1],
                       w0_full[2 * j], w0_full[2 * j + 1]]
        blocks += [at_sl[6], w0_full[6]]
        aw_packs.append(np.ascontiguousarray(
            np.concatenate(blocks, axis=1)).astype(f8))

    in_maps = []
    for c in range(N_CORES):
        sg, tg = c // TG, c % TG
        in_maps.append({
            "aw_pack": aw_packs[tg],
            "w1_pack": w1_packs[sg],
            "w2_pack": w2_pack,
        })

    if "nc" not in _CACHE:
        _CACHE["nc"] = _build_program()
    nc = _CACHE["nc"]

    trace = bool(int(os.environ.get("KERNEL_TRACE", "0")))
    res = run_bass_kernel_spmd(
        nc, in_maps, core_ids=list(range(N_CORES)), trace=trace)
    last_results = res

    out = np.empty((S, BT, D_OUT), np.float32)
    for c in range(N_CORES):
        sg, tg = c // TG, c % TG
        zt = np.asarray(res.results[c]["zt"], dtype=np.float32)  # [10, 8*512]
        for i in range(R_LOC):
            logits = zt[:, i * NT:(i + 1) * NT].T                # [512, 10]
            out[sg * R_LOC + i, tg * NT:(tg + 1) * NT] = (
                1.0 / (1.0 + np.exp(-logits)))
    return out.reshape(S, 32, 64, D_OUT)


# revision 36
# speedup vs baseline: 1.1266x; 1.0206x over previous
"""Trainium2 Bass kernel for the Noisy-Weights BNN MLP.

Computation (full problem):
  noise1[0] = 0;  W1n = W1[None] + noise1            # [16, 512, 512]
  X = sigmoid(A @ W0)        A = batch.reshape(2048, 784)
  Y_s = sigmoid(X @ W1n[s])
  Z_s = sigmoid(Y_s @ W2)    -> out [16, 32, 64, 10]

Sharding over 8 NeuronCores: 2 replica-groups (8 replicas each) x
4 token-groups (512 tokens each).  Each core redundantly computes the
shared layer 0 for its 512 tokens, then its 8 replicas of layers 1+2.

On-device layout: every matmul is a native out = lhsT.T @ rhs with the
contraction dim on SBUF partitions:
  layer0: lhsT = W0 tile, rhs = A^T tile -> psum X^T, sigmoid -> fp8
  layer1: lhsT = W1n tile, rhs = X^T     -> psum Y^T, sigmoid -> fp8
  layer2: lhsT = W2 tile [128k, 16pad], rhs = Y^T, two DoubleRow
          k-pair matmuls -> psum Z^T logits, one DVE copy to SBUF
Host applies the final sigmoid + transpose (tiny: 1.3 MB total).

Precision: all three layers run fp8e4m3 with DoubleRow perf mode
(2 k-tiles per matmul, 2x PE throughput when warm, half the DMA
bytes).  numpy-model rel-L2 vs the fp32 reference: 1.0e-2 (gate is
2e-2).  fp32 PSUM accumulation throughout.

Schedule notes: a few N=512 dummy matmuls warm the PE clock (HAM)
while the first DMA chunk lands; layer-0 A^T/W0 are packed in k-tile
pair blocks and DMA'd in chunks so compute starts after the first
256 KB; each replica's layer-2 k-pairs are issued one sigmoid behind
(pair mp in slot mp of the NEXT replica) so the PE FIFO never stalls
behind a not-yet-ready activation.  Steady state runs both TensorE
and ScalarE at ~100% occupancy (~2.0 us per replica).
"""

import os
import sys

import numpy as np
import ml_dtypes

if "/opt/trn_rl_repo" not in sys.path:
    sys.path.insert(0, "/opt/trn_rl_repo")

import concourse.bass as bass  # noqa: E402
import concourse.tile as tile  # noqa: E402
from concourse import bacc, mybir  # noqa: E402
from concourse.bass_utils import run_bass_kernel_spmd  # noqa: E402

# ---- problem constants (hardcoded; kernel.py must be self-contained) ----
S = 16           # noisy-weight replicas
BT = 2048        # batch tokens = 32 * 64
D_IN = 784
D_H = 512
D_OUT = 10
KA = 896         # 784 zero-padded to 7 * 128
N_CORES = 8
SG = 2           # replica groups
TG = 4           # token groups
R_LOC = S // SG          # replicas per core = 8
NT = BT // TG            # tokens per core = 512
KA_T = KA // 128         # 7 k-tiles for layer 0
KH_T = D_H // 128        # 4 k-tiles / m-tiles for hidden dims
AW_K = NT + D_H          # interleaved A^T|W0 stride per k-tile = 1024
W2C = 16                 # W2 k-tile columns: 10 outputs padded to 16 so the
                         # DoubleRow weight AP stride is 16 B-aligned

BF16 = mybir.dt.bfloat16
FP8 = mybir.dt.float8e4
F32 = mybir.dt.float32
DR = mybir.MatmulPerfMode.DoubleRow

# Dummy matmuls covering the first input-DMA wait (~7.5us -> ~10.4us: DMA
# can't start before the NEFF preamble ends and its completion semaphore
# takes ~0.75us after the data lands).  Keeping the PE busy the whole time
# both avoids the idle gap and lets the HAM clock gate reach 2.4 GHz
# before the first real matmul — cold DR matmuls are 2x slower.  N=512
# streaming matmuls (~85% duty) are needed to register as "busy" with
# the HAM activity window; short N=128 ones (~50% duty) leave it cold.
# The HAM window is free-running, so a minimal 3.4us burst only fires
# the warm transition on lucky alignment (~50% of runs measured 39.3-
# 40.4us, the rest 42.4-43.8).  11 matmuls (~4.7us) make warm-by-
# layer-0 nearly deterministic: runs measure a stable ~40.5-41.3 with
# no slow tail.  Warmup time and cold-running time trade ~1:1, so this
# costs nothing in expectation and removes the variance.
N_WARM = 11

_CACHE = {}

last_results = None  # BassKernelResults of the most recent run (for test.py)


def _build_program():
    """One SPMD Bass program; per-core differences live entirely in data."""
    nc = bacc.Bacc(None, target_bir_lowering=False, debug=False,
                   enable_partition_id=False)

    # layer-0 inputs interleaved per k-tile: aw[:, k*1024+0:512] = A^T k-tile,
    # aw[:, k*1024+512:1024] = W0 k-tile
    aw_d = nc.dram_tensor("aw_pack", [128, KA_T * AW_K], FP8,
                          kind="ExternalInput")
    w1_d = nc.dram_tensor("w1_pack", [128, R_LOC * KH_T * D_H], FP8,
                          kind="ExternalInput")
    w2_d = nc.dram_tensor("w2_pack", [128, KH_T * W2C], FP8,
                          kind="ExternalInput")
    zt_d = nc.dram_tensor("zt", [D_OUT, R_LOC * NT], F32, kind="ExternalOutput")

    SIG = mybir.ActivationFunctionType.Sigmoid
    AW_CHUNKS = [(0, 2), (2, 4), (4, 6)]   # full k-tile ranges per chunk
    K6 = (KA_T - 1) * AW_K                 # col offset of the 16-row k-tile 6

    with tile.TileContext(nc) as tc:
        with (
            tc.tile_pool(name="consts", bufs=1) as consts,
            tc.tile_pool(name="w1p", bufs=1) as w1p,
            tc.tile_pool(name="yp", bufs=3) as yp,
            tc.tile_pool(name="px", bufs=3, space="PSUM") as px,
            tc.tile_pool(name="pz", bufs=2, space="PSUM") as pz,
        ):
            warm_sb = consts.tile([128, 512], BF16)
            aw_sb = consts.tile([128, KA_T * AW_K], FP8)
            w2_sb = consts.tile([128, KH_T * W2C], FP8)
            x_sb = consts.tile([128, KH_T * NT], FP8)
            z_sb = consts.tile([D_OUT, R_LOC * NT], F32)

            # PE warm-up: dummy matmuls keep TensorE busy (and un-throttle
            # the HAM clock gate) while the first input DMA lands.
            nc.vector.memset(warm_sb[:], 0)
            wps = px.tile([128, 1024], F32, name="ps")
            for _ in range(N_WARM):
                nc.tensor.matmul(wps[:, :512], lhsT=warm_sb[:, :128],
                                 rhs=warm_sb[:], start=True, stop=True)

            # Input DMA order is the critical path: the load phase is
            # HBM-bandwidth-bound (~330 GB/s aggregate), so order transfers
            # by when compute first needs them.  k-tile 6 holds only 16
            # valid rows (784 = 6*128 + 16) — transfer just those
            # partitions, and put it first so the layer-0 accumulation can
            # OPEN with it and close on the last full chunk.
            nc.sync.dma_start(out=aw_sb[:, 0:2 * AW_K],
                              in_=aw_d[:, 0:2 * AW_K])
            nc.sync.dma_start(out=aw_sb[0:16, K6:K6 + AW_K],
                              in_=aw_d[0:16, K6:K6 + AW_K])
            for k0, k1 in AW_CHUNKS[1:]:
                nc.sync.dma_start(
                    out=aw_sb[:, k0 * AW_K:k1 * AW_K],
                    in_=aw_d[:, k0 * AW_K:k1 * AW_K])
            # replica 0's weights right after the aw chunks (its layer 1
            # starts ~3us before any other replica's), then singles/pairs
            # in consumption order.
            RW = KH_T * D_H
            W1_CHUNKS = [(0, 1), (1, 2), (2, 4), (4, 6), (6, 8)]
            w1_sb = [(c0, w1p.tile([128, (c1 - c0) * RW], FP8,
                                   name=f"w1c{ci}"))
                     for ci, (c0, c1) in enumerate(W1_CHUNKS)]
            nc.sync.dma_start(out=w1_sb[0][1][:], in_=w1_d[:, 0:RW])
            nc.sync.dma_start(out=w2_sb[:], in_=w2_d[:])
            for ci in range(1, 5):
                c0, c1 = W1_CHUNKS[ci]
                nc.sync.dma_start(out=w1_sb[ci][1][:],
                                  in_=w1_d[:, c0 * RW:c1 * RW])

            # ---- layer 0: X^T = sigmoid(W0^T A^T), fp8 DoubleRow ----
            # The aw pack stores k-tile PAIR blocks [AT_k|AT_k+1|W0_k|
            # W0_k+1] so both DoubleRow operands have a contiguous 512 B
            # k-plane stride — DoubleRow only streams 2 fp8/cycle (216 ns
            # vs 427 ns per mm) when the two k-planes are adjacent in the
            # partition line.  k-outer within each m-pair so early chunks
            # are consumed while later chunks are still in flight.
            for mp in range(2):           # m pairs: (0,1), (2,3)
                ps = px.tile([128, 1024], F32, name="ps")
                # the 16-row k-tile 6 opens the accumulation (its DMA is
                # tiny and early), so each m-pair closes as soon as the
                # LAST full chunk lands — the sigmoid isn't pushed behind
                # a straggler tile.
                for m2 in range(2):
                    m = 2 * mp + m2
                    nc.tensor.matmul(
                        ps[:, m2 * NT:(m2 + 1) * NT],
                        lhsT=aw_sb[0:16, K6 + NT + m * 128:
                                   K6 + NT + (m + 1) * 128],
                        rhs=aw_sb[0:16, K6:K6 + NT],
                        start=True, stop=False,
                    )
                for j in range(3):        # k-tile pairs (0,1), (2,3), (4,5)
                    blk = j * 2 * AW_K
                    at2 = aw_sb[:, blk:blk + 2 * NT].rearrange(
                        "p (q n) -> p q n", q=2)
                    w02 = aw_sb[:, blk + 2 * NT:blk + 2 * AW_K].rearrange(
                        "p (q n) -> p q n", q=2)
                    for m2 in range(2):
                        m = 2 * mp + m2
                        nc.tensor.matmul(
                            ps[:, m2 * NT:(m2 + 1) * NT],
                            lhsT=w02[:, :, m * 128:(m + 1) * 128],
                            rhs=at2[:],
                            start=False, stop=(j == 2),
                            perf_mode=DR,
                        )
                # one sigmoid per m-pair: finer [128,512] splits measure
                # WORSE (45.5us vs 39.6) — the scheduler's counting-
                # semaphore waits get assigned to later PE positions and
                # delay the whole ACT stream.
                nc.scalar.activation(
                    x_sb[:, mp * 1024:(mp + 1) * 1024], ps[:], SIG)

            # ---- per replica: layer 1 (fp8 DoubleRow), layer 2 deferred ----
            x3 = x_sb[:].rearrange("p (k n) -> p k n", k=KH_T)
            y_sbs = {}
            psz = {}

            def w1c3_of(r):
                for c0, w1c in reversed(w1_sb):
                    if r >= c0:
                        roff = (r - c0) * RW
                        return w1c[:, roff:roff + RW].rearrange(
                            "p (k n) -> p k n", k=KH_T)
                raise AssertionError(r)

            def l1_mm(ps, w13, mp, m2, kp):
                m = 2 * mp + m2
                nc.tensor.matmul(
                    ps[:, m2 * NT:(m2 + 1) * NT],
                    lhsT=w13[:, kp:kp + 2, m * 128:(m + 1) * 128],
                    rhs=x3[:, kp:kp + 2, :],
                    start=(kp == 0), stop=(kp == 2),
                    perf_mode=DR,
                )

            w23 = w2_sb[:].rearrange("p (k n) -> p k n", k=KH_T)

            def l2_pair(r, mp):
                # one DoubleRow matmul covers a k-tile pair; both pairs
                # accumulate into the same PSUM partials [0:16].
                y3r = y_sbs[r][:].rearrange("p (k n) -> p k n", k=KH_T)
                nc.tensor.matmul(
                    psz[r][0:W2C, :],
                    lhsT=w23[:, 2 * mp:2 * mp + 2, :],
                    rhs=y3r[:, 2 * mp:2 * mp + 2, :],
                    start=(mp == 0), stop=(mp == 1),
                    perf_mode=DR,
                )

            def l2_reduce(r):
                # single accumulation group -> one PSUM->SBUF copy
                nc.vector.tensor_copy(out=z_sb[:, r * NT:(r + 1) * NT],
                                      in_=psz[r][0:D_OUT, :])
                psz.pop(r)

            # replica 0: kp-outer across BOTH m-pair psums — its second
            # k-pair is gated on layer 0's second sigmoid, so consume kp0
            # for all four m-tiles first.
            w13 = w1c3_of(0)
            y_sbs[0] = yp.tile([128, KH_T * NT], FP8, name="y_sb")
            psz[0] = pz.tile([128, NT], F32, name="psz")
            ps_r0 = [px.tile([128, 1024], F32, name="ps") for _ in range(2)]
            for kp in (0, 2):
                for mp in range(2):
                    for m2 in range(2):
                        l1_mm(ps_r0[mp], w13, mp, m2, kp)
            for mp in range(2):
                nc.scalar.activation(
                    y_sbs[0][:, mp * 1024:(mp + 1) * 1024], ps_r0[mp][:], SIG)

            for r in range(1, R_LOC):
                w13 = w1c3_of(r)
                y_sbs[r] = yp.tile([128, KH_T * NT], FP8, name="y_sb")
                psz[r] = pz.tile([128, NT], F32, name="psz")
                last = (r == R_LOC - 1)
                for mp in range(2):
                    ps = px.tile([128, 1024], F32, name="ps")
                    for m2 in range(2):
                        for kp in (0, 2):
                            l1_mm(ps, w13, mp, m2, kp)
                    if last and mp == 1:
                        # last replica: split the final sigmoid so its
                        # layer 2 can start after the first half
                        nc.scalar.activation(
                            y_sbs[r][:, 1024:1536], ps[:, :512], SIG)
                        nc.scalar.activation(
                            y_sbs[r][:, 1536:2048], ps[:, 512:], SIG)
                    else:
                        nc.scalar.activation(
                            y_sbs[r][:, mp * 1024:(mp + 1) * 1024], ps[:], SIG)
                    if last:
                        if mp == 0:
                            # r6's layer 2 entirely here so its DVE reduce
                            # and the z[4:7] DMA clear out before the tail
                            l2_pair(r - 1, 0)
                            l2_pair(r - 1, 1)
                            l2_reduce(r - 1)
                            y_sbs.pop(r - 1)
                            nc.sync.dma_start(
                                out=zt_d[:, 4 * NT:7 * NT],
                                in_=z_sb[:, 4 * NT:7 * NT])
                    else:
                        # split-slot injection: pair mp of the PREVIOUS
                        # replica's layer 2 goes in slot mp, one sigmoid
                        # behind — its y-halves are guaranteed ready
                        l2_pair(r - 1, mp)
                        if mp == 1:
                            l2_reduce(r - 1)
                            y_sbs.pop(r - 1)
                            if r == 4:
                                nc.sync.dma_start(
                                    out=zt_d[:, :4 * NT], in_=z_sb[:, :4 * NT])

            # last replica's layer 2: the (k0,k1) DoubleRow pair needs only
            # the mp0 sigmoid; k2/k3 run as normal matmuls so each can
            # start right after its half of the split mp1 sigmoid.
            r = R_LOC - 1
            nc.tensor.matmul(
                psz[r][0:W2C, :],
                lhsT=w23[:, 0:2, :],
                rhs=y_sbs[r][:].rearrange("p (k n) -> p k n",
                                          k=KH_T)[:, 0:2, :],
                start=True, stop=False, perf_mode=DR,
                skip_group_check=True,
            )
            for k in (2, 3):
                nc.tensor.matmul(
                    psz[r][0:D_OUT, :],
                    lhsT=w2_sb[:, k * W2C:k * W2C + D_OUT],
                    rhs=y_sbs[r][:, k * NT:(k + 1) * NT],
                    start=False, stop=(k == 3),
                    skip_group_check=True,
                )
            # single copy + single DMA: one HBM write receipt on the tail
            nc.vector.tensor_copy(out=z_sb[:, r * NT:(r + 1) * NT],
                                  in_=psz[r][0:D_OUT, :])
            nc.sync.dma_start(out=zt_d[:, 7 * NT:8 * NT],
                              in_=z_sb[:, 7 * NT:8 * NT])

    nc.compile()
    return nc


def _pack_kxm(w, ktiles):
    """[K, M] -> [128, ktiles*M] with pack[p, k*M + m] = w[k*128 + p, m]."""
    K, M = w.shape
    assert K == ktiles * 128
    return np.ascontiguousarray(
        w.reshape(ktiles, 128, M).transpose(1, 0, 2).reshape(128, ktiles * M)
    )


def kernel(batch, W0, W1, W2, noise1):
    global last_results
    batch = np.asarray(batch, dtype=np.float32)
    W0 = np.asarray(W0, dtype=np.float32)
    W1 = np.asarray(W1, dtype=np.float32)
    W2 = np.asarray(W2, dtype=np.float32)
    noise1 = np.asarray(noise1, dtype=np.float32)

    bf = ml_dtypes.bfloat16
    f8 = mybir.dt.np(FP8)

    A = batch.reshape(BT, D_IN)
    ATp = np.zeros((KA, BT), np.float32)
    ATp[:D_IN] = A.T
    at_full = ATp.reshape(KA_T, 128, BT)          # [k, p, n]

    W0p = np.zeros((KA, D_H), np.float32)
    W0p[:D_IN] = W0
    w0_full = W0p.reshape(KA_T, 128, D_H)         # [k, p, m]

    noise = noise1.copy()
    noise[0] = 0.0
    W1n = W1[None] + noise                        # [16, 512, 512] fp32

    W2p = np.zeros((D_H, W2C), np.float32)
    W2p[:, :D_OUT] = W2
    w2_pack = _pack_kxm(W2p, KH_T).astype(f8)

    # per-replica-group W1 packs: [p, (r k n)]
    w1_packs = []
    for sg in range(SG):
        blk = W1n[sg * R_LOC:(sg + 1) * R_LOC]    # [8, 512, 512]
        p = blk.reshape(R_LOC, KH_T, 128, D_H).transpose(2, 0, 1, 3)
        w1_packs.append(np.ascontiguousarray(
            p.reshape(128, R_LOC * KH_T * D_H)).astype(f8))

    # per-token-group A^T|W0 packs in k-tile PAIR blocks:
    # [AT_2j | AT_2j+1 | W0_2j | W0_2j+1] x3, then [AT_6 | W0_6]
    aw_packs = []
    for tg in range(TG):
        at_sl = at_full[:, :, tg * NT:(tg + 1) * NT]      # [k, p, 512]
        blocks = []
        for j in range(3):
            blocks += [at_sl[2 * j], at_sl[2 * j + 1],
                       w0_full[2 * j], w0_full[2 * j + 1]]
        blocks += [at_sl[6], w0_full[6]]
        aw_packs.append(np.ascontiguousarray(
            np.concatenate(blocks, axis=1)).astype(f8))

    in_maps = []
    for c in range(N_CORES):
        sg, tg = c // TG, c % TG
        in_maps.append({
            "aw_pack": aw_packs[tg],
            "w1_pack": w1_packs[sg],
            "w2_pack": w2_pack,
        })

    if "nc" not in _CACHE:
        _CACHE["nc"] = _build_program()
    nc = _CACHE["nc"]

    trace = bool(int(os.environ.get("KERNEL_TRACE", "0")))
    res = run_bass_kernel_spmd(
        nc, in_maps, core_ids=list(range(N_CORES)), trace=trace)
    last_results = res

    out = np.empty((S, BT, D_OUT), np.float32)
    for c in range(N_CORES):
        sg, tg = c // TG, c % TG
        zt = np.asarray(res.results[c]["zt"], dtype=np.float32)  # [10, 8*512]
        for i in range(R_LOC):
            logits = zt[:, i * NT:(i + 1) * NT].T                # [512, 10]
            out[sg * R_LOC + i, tg * NT:(tg + 1) * NT] = (
                1.0 / (1.0 + np.exp(-logits)))
    return out.reshape(S, 32, 64, D_OUT)
